# revision 1
# baseline (speedup 1.0000x reference)
"""Trainium2 Bass kernel for the MinimalLRU forward pass.

Strategy (8 NeuronCores, data-parallel over batch, one row per core):

  reference math per (b):   u[t, c]   = x[t, :] @ W_in.T + b_in          (complex c = re|im planes)
                            h[t, c]   = lam_c * h[t-1, c] + u[t, c]      (complex diagonal scan)
                            out[t, s] = [Re h, Im h] @ W_out.T + b_out, then L2-normalized over s

  Key trick: lam_c = r_c * exp(i*theta_c). Factor the complex scan as
      h[c, t] = exp(i*theta_c*t) * S[c, t]
      S[c, t] = r_c * S[c, t-1] + exp(-i*theta_c*t) * u[c, t]
  The remaining recurrence has a REAL per-channel multiplier r_c, so it splits
  into two independent real scans (re/im planes) that map directly onto the
  hardware `tensor_tensor_scan` instruction ([channel=partition, time=free]).
  The phase factors exp(+-i*theta_c*t) are elementwise rotations against
  host-precomputed cos/sin tables.

  Per-core pipeline (channel-on-partition layout [128, t] everywhere):
    DMA xT span -> PE mm1 (W_inT stationary, xT moving) -> PSUM u planes
    -> ACT copy psum->sbuf f16 -> DVE E- rotation -> DVE real scans (carried
    state across spans) -> DVE/GpSimd E+ rotation -> PE mm2 per 128-t tile
    -> ACT square+accum / sqrt, DVE max/recip -> scaled copy -> DMA out.

  All matmul operands and elementwise tensors are fp16 (fp32 accumulate in
  PSUM and fp32 scan state), output fp32.
"""

import sys

import numpy as np

sys.path.insert(0, "/opt/trn_rl_repo")

import concourse.bass as bass  # noqa: E402
import concourse.tile as tile  # noqa: E402
from concourse import mybir  # noqa: E402
from concourse.bass_utils import run_bass_kernel_spmd  # noqa: E402

F16 = mybir.dt.float16
F32 = mybir.dt.float32


def _legalize_waits(nc):
    """The walrus in this container accepts at most ONE sync wait per
    instruction. Post-finalize, hoist extra waits onto preceding
    single-wait NOPs on the same engine (engine dispatch is in-order, so
    the instruction still starts only after all original waits clear)."""
    import bass_rust

    for fnc in nc.m.functions:
        for blk in fnc.blocks:
            insts = list(blk.instructions)
            changed = False
            out = []
            for inst in insts:
                si = inst.sync_info
                if si is not None and len(si.on_wait) > 1:
                    waits = list(si.on_wait)
                    for j, w in enumerate(waits[:-1]):
                        d = mybir.InstNoOp(
                            name=f"{inst.name}-w{j}",
                            text_hint="waitsplit",
                            bass_nofuse=True,
                            sync_info=bass_rust.SyncInfo(
                                on_wait=[w], on_update=[]
                            ),
                        )
                        d.engine = inst.engine
                        out.append(d)
                    inst.sync_info = bass_rust.SyncInfo(
                        on_wait=[waits[-1]], on_update=list(si.on_update)
                    )
                    changed = True
                out.append(inst)
            if changed:
                blk.instructions = out
AF = mybir.ActivationFunctionType
OP = mybir.AluOpType

TOKEN_DIM = 512
STATE_DIM = 256
HIDDEN = 128
B_FULL = 8
T_FULL = 8192
SPAN = 512  # timesteps per pipeline stage (one PSUM bank per u plane)
N_CORES = 8


def build_nc(T=T_FULL, span=SPAN):
    """Build the single-core Bass program (same NEFF runs SPMD on all cores)."""
    assert T % span == 0 and span % 128 == 0
    n_spans = T // span
    tt_per_span = span // 128
    DC = TOKEN_DIM // 128  # d-chunks for mm1 contraction

    nc = bass.Bass(trn_type="TRN2", debug=False)

    xt = nc.dram_tensor("xt", [TOKEN_DIM, T], F16, kind="ExternalInput")
    tcos = nc.dram_tensor("tcos", [HIDDEN, T], F16, kind="ExternalInput")
    tsin = nc.dram_tensor("tsin", [HIDDEN, T], F16, kind="ExternalInput")
    dec = nc.dram_tensor("dec", [HIDDEN, span], F16, kind="ExternalInput")
    winT = nc.dram_tensor("winT", [128, DC, 2 * HIDDEN], F16, kind="ExternalInput")
    wot = nc.dram_tensor("wot", [128, 2, STATE_DIM], F16, kind="ExternalInput")
    bin2 = nc.dram_tensor("bin2", [1, 2 * HIDDEN], F16, kind="ExternalInput")
    bout1 = nc.dram_tensor("bout1", [1, STATE_DIM], F16, kind="ExternalInput")
    out = nc.dram_tensor("out", [T, STATE_DIM], F32, kind="ExternalOutput")

    with tile.TileContext(nc) as tc:
        with (
            tc.tile_pool(name="singles", bufs=1) as singles,
            tc.tile_pool(name="xq", bufs=3) as xq_pool,
            tc.tile_pool(name="usb", bufs=2) as usb_pool,
            tc.tile_pool(name="rot", bufs=2) as rot_pool,
            tc.tile_pool(name="scan", bufs=3) as scan_pool,
            tc.tile_pool(name="feat", bufs=2) as feat_pool,
            tc.tile_pool(name="outsb", bufs=4) as out_pool,
            tc.tile_pool(name="stat", bufs=8) as stat_pool,
            tc.tile_pool(name="psum_u", bufs=2, space="PSUM") as psum_u_pool,
            tc.tile_pool(name="psum_o", bufs=4, space="PSUM") as psum_o_pool,
        ):
            # --- constants, loaded once ---
            # per-span table tiles: tile-granular dep tracking means a single
            # big table tile written by many DMAs would make every consumer
            # wait on all of them (walrus "too many sync waits")
            cos_t = [singles.tile([HIDDEN, span], F16, name=f"cos{q}",
                                  tag=f"cos{q}") for q in range(n_spans)]
            sin_t = [singles.tile([HIDDEN, span], F16, name=f"sin{q}",
                                  tag=f"sin{q}") for q in range(n_spans)]
            dec_sb = singles.tile([HIDDEN, span], F16, tag="dec")
            win_sb = singles.tile([128, DC, 2 * HIDDEN], F16, tag="win")
            wot_sb = singles.tile([128, 2, STATE_DIM], F16, tag="wot")
            bin_sb = singles.tile([1, 2 * HIDDEN], F16, tag="bin")
            bout_sb = singles.tile([1, STATE_DIM], F16, tag="bout")
            ones_sp = singles.tile([1, span], F16, tag="ones_sp")
            ones_tt = singles.tile([1, 128], F16, tag="ones_tt")

            for q in range(n_spans):
                sl = slice(q * span, (q + 1) * span)
                nc.sync.dma_start(out=cos_t[q], in_=tcos[:, sl])
                nc.sync.dma_start(out=sin_t[q], in_=tsin[:, sl])
            nc.sync.dma_start(out=dec_sb, in_=dec[:, :])
            nc.sync.dma_start(out=win_sb, in_=winT[:, :, :])
            nc.sync.dma_start(out=wot_sb, in_=wot[:, :, :])
            nc.sync.dma_start(out=bin_sb, in_=bin2[:, :])
            nc.sync.dma_start(out=bout_sb, in_=bout1[:, :])
            nc.vector.memset(ones_sp, 1.0)
            nc.vector.memset(ones_tt, 1.0)

            carry_re = None  # AP of previous span's last scan column
            carry_im = None

            for q in range(n_spans):
                t0 = q * span
                # --- load x^T span: 4 d-chunks of [128, span] ---
                xq = xq_pool.tile([128, DC, span], F16, tag="xq")
                nc.sync.dma_start(
                    out=xq,
                    in_=xt[:, t0 : t0 + span].rearrange(
                        "(a p) t -> p a t", p=128
                    ),
                )

                # --- mm1: u planes [c=128, t=span] in PSUM ---
                u_sb = usb_pool.tile([128, 2, span], F16, tag="usb")
                for plane in range(2):
                    psum_u = psum_u_pool.tile(
                        [128, span], F32, tag=f"psum_u{plane}"
                    )
                    nc.tensor.matmul(
                        psum_u,
                        lhsT=bin_sb[:, plane * 128 : (plane + 1) * 128],
                        rhs=ones_sp,
                        start=True,
                        stop=False,
                    )
                    for dc in range(DC):
                        nc.tensor.matmul(
                            psum_u,
                            lhsT=win_sb[:, dc, plane * 128 : (plane + 1) * 128],
                            rhs=xq[:, dc, :],
                            start=False,
                            stop=(dc == DC - 1),
                        )
                    # psum f32 -> sbuf f16 (ScalarE, near PSUM)
                    nc.scalar.copy(u_sb[:, plane, :], psum_u)

                c_sp = cos_t[q][:, :]
                s_sp = sin_t[q][:, :]

                # --- E- rotation (DVE):  ur = C*u_re + S*u_im ; ui = C*u_im - S*u_re
                m1 = rot_pool.tile([128, span], F16, tag="m1")
                m2 = rot_pool.tile([128, span], F16, tag="m2")
                ur = rot_pool.tile([128, span], F16, tag="ur")
                ui = rot_pool.tile([128, span], F16, tag="ui")
                nc.vector.tensor_mul(m1, c_sp, u_sb[:, 0, :])
                nc.vector.tensor_mul(m2, s_sp, u_sb[:, 1, :])
                nc.vector.tensor_add(ur, m1, m2)
                nc.vector.tensor_mul(m1, c_sp, u_sb[:, 1, :])
                nc.vector.tensor_mul(m2, s_sp, u_sb[:, 0, :])
                nc.vector.tensor_sub(ui, m1, m2)

                # --- real scans with carried state (DVE) ---
                s_re = scan_pool.tile([128, span], F16, tag="s_re")
                s_im = scan_pool.tile([128, span], F16, tag="s_im")
                nc.vector.tensor_tensor_scan(
                    s_re, dec_sb, ur,
                    0.0 if carry_re is None else carry_re,
                    op0=OP.mult, op1=OP.add,
                )
                nc.vector.tensor_tensor_scan(
                    s_im, dec_sb, ui,
                    0.0 if carry_im is None else carry_im,
                    op0=OP.mult, op1=OP.add,
                )
                carry_re = s_re[:, span - 1 : span]
                carry_im = s_im[:, span - 1 : span]

                # --- E+ rotation: feat_re = C*S_re - S*S_im (GpSimd)
                #                  feat_im = C*S_im + S*S_re (DVE)
                g1 = rot_pool.tile([128, span], F16, tag="g1")
                g2 = rot_pool.tile([128, span], F16, tag="g2")
                feat_re = feat_pool.tile([128, span], F16, tag="feat_re")
                feat_im = feat_pool.tile([128, span], F16, tag="feat_im")
                nc.gpsimd.tensor_mul(g1, c_sp, s_re)
                nc.gpsimd.tensor_mul(g2, s_sp, s_im)
                nc.gpsimd.tensor_sub(feat_re, g1, g2)
                g3 = rot_pool.tile([128, span], F16, tag="g3")
                g4 = rot_pool.tile([128, span], F16, tag="g4")
                nc.vector.tensor_mul(g3, c_sp, s_im)
                nc.vector.tensor_mul(g4, s_sp, s_re)
                nc.vector.tensor_add(feat_im, g3, g4)

                # --- mm2 + norm per 128-t tile ---
                for tt in range(tt_per_span):
                    j0 = tt * 128
                    psum_o = psum_o_pool.tile([128, STATE_DIM], F32, tag="psum_o")
                    nc.tensor.matmul(
                        psum_o, lhsT=ones_tt, rhs=bout_sb, start=True, stop=False
                    )
                    nc.tensor.matmul(
                        psum_o,
                        lhsT=feat_re[:, j0 : j0 + 128],
                        rhs=wot_sb[:, 0, :],
                        start=False,
                        stop=False,
                    )
                    nc.tensor.matmul(
                        psum_o,
                        lhsT=feat_im[:, j0 : j0 + 128],
                        rhs=wot_sb[:, 1, :],
                        start=False,
                        stop=True,
                    )
                    sq = stat_pool.tile([128, STATE_DIM], F16, tag="sq")
                    ss = stat_pool.tile([128, 1], F32, tag="ss")
                    nc.scalar.activation(sq, psum_o, AF.Square, accum_out=ss)
                    nrm = stat_pool.tile([128, 1], F32, tag="nrm")
                    nc.scalar.activation(nrm, ss, AF.Sqrt)
                    nrm2 = stat_pool.tile([128, 1], F32, tag="nrm2")
                    nc.vector.tensor_scalar_max(nrm2, nrm, 1e-12)
                    rcp = stat_pool.tile([128, 1], F32, tag="rcp")
                    nc.vector.reciprocal(rcp, nrm2)
                    o_sb = out_pool.tile([128, STATE_DIM], F32, tag="o_sb")
                    nc.vector.tensor_scalar(
                        o_sb, psum_o, rcp, None, op0=OP.mult
                    )
                    nc.gpsimd.dma_start(
                        out=out[t0 + j0 : t0 + j0 + 128, :], in_=o_sb
                    )
    nc.finalize()
    _legalize_waits(nc)
    return nc


def _host_inputs(x, W_in, b_in, log_radius, phase, W_out, b_out, T, span):
    """Per-core input maps (core b <- batch row b)."""
    H = HIDDEN
    radius = 1.0 / (1.0 + np.exp(-np.asarray(log_radius, np.float64)))
    theta = np.asarray(phase, np.float64)
    ang = np.outer(theta, np.arange(T, dtype=np.float64))
    tcos = np.cos(ang).astype(np.float16)
    tsin = np.sin(ang).astype(np.float16)
    dec = np.ascontiguousarray(
        np.broadcast_to(radius.astype(np.float16)[:, None], (H, span))
    )
    winT = np.ascontiguousarray(
        W_in.T.reshape(TOKEN_DIM // 128, 128, 2 * H).transpose(1, 0, 2)
    ).astype(np.float16)
    wot = np.ascontiguousarray(
        W_out.T.reshape(2, 128, STATE_DIM).transpose(1, 0, 2)
    ).astype(np.float16)
    bin2 = np.ascontiguousarray(b_in.reshape(1, 2 * H)).astype(np.float16)
    bout1 = np.ascontiguousarray(b_out.reshape(1, STATE_DIM)).astype(np.float16)

    shared = dict(tcos=tcos, tsin=tsin, dec=dec, winT=winT, wot=wot,
                  bin2=bin2, bout1=bout1)
    in_maps = []
    B = x.shape[0]
    for b in range(B):
        xt = np.ascontiguousarray(x[b, :T].T).astype(np.float16)
        in_maps.append(dict(shared, xt=xt))
    return in_maps


_NC_CACHE = {}


def run(x, W_in, b_in, log_radius, phase, W_out, b_out, T=T_FULL, span=SPAN,
        **spmd_kwargs):
    key = (T, span)
    if key not in _NC_CACHE:
        _NC_CACHE[key] = build_nc(T, span)
    nc = _NC_CACHE[key]
    in_maps = _host_inputs(x, W_in, b_in, log_radius, phase, W_out, b_out, T, span)
    res = run_bass_kernel_spmd(nc, in_maps, core_ids=list(range(len(in_maps))),
                               **spmd_kwargs)
    outs = np.stack([r["out"] for r in res.results], 0)
    return outs, res


def kernel(x, mask, W_in, b_in, log_radius, phase, W_out, b_out):
    # mask is all-ones per the problem spec; the recurrence treats every
    # timestep as valid.
    outs, _ = run(x, W_in, b_in, log_radius, phase, W_out, b_out)
    return outs.astype(np.float32)


if __name__ == "__main__":
    nc = build_nc(1024, SPAN)
    print("built ok")

